# revision 1
# baseline (speedup 1.0000x reference)
"""GCN aggregator kernel for Trainium2 (Bass/Tile), 8-core data-parallel.

Computes: out = relu(((sum_k neigh[:,k,:] + self) / (K+1)) @ W + b)
Sharding: nodes (N) split evenly across 8 NeuronCores; W/b replicated.

Per 128-node tile on each core:
  1. DMA neigh tile [128, K*D] + self tile [128, D]           (sync HWDGE)
  2. DVE reduce_sum over k (strided AP) + add self            (VectorE)
  3. PE transpose sum -> sumT in PSUM, ACT copy w/ 1/(K+1)    (TensorE/ScalarE)
  4. PE GEMM sumT.T @ W accumulated over 4 d-chunks + bias    (TensorE)
  5. ACT relu PSUM->SBUF, DMA store                           (ScalarE HWDGE)
"""

import os
import sys

import numpy as np

for _p in ("/opt/trn_rl_repo", "/root/.axon_site/_ro/trn_rl_repo"):
    if os.path.isdir(_p) and _p not in sys.path:
        sys.path.insert(0, _p)

import concourse.bass as bass
import concourse.tile as tile
from concourse import bacc, mybir
from concourse.masks import make_identity

N, K, D, O = 16384, 25, 512, 1024
N_CORES = 8
P = 128  # nodes per tile (partition count)
INV = 1.0 / (K + 1)
FP = mybir.dt.float32


def _tree_fold(nc, t, g):
    """In-place pairwise fold of `g` contiguous D-sized groups in tile t;
    result lands in t[:, :D]."""
    while g > 1:
        lo = g // 2
        nc.vector.tensor_add(
            t[:, : lo * D], t[:, : lo * D], t[:, (g - lo) * D : g * D]
        )
        g -= lo


def build_nc(n_nodes: int, neigh_bufs: int = 3) -> bass.Bass:
    """Build the per-core Bass program for a shard of `n_nodes` nodes."""
    assert n_nodes % P == 0
    nt = n_nodes // P

    nc = bacc.Bacc("TRN2", target_bir_lowering=False, debug=False)
    self_h = nc.dram_tensor("self_vecs", [n_nodes, D], FP, kind="ExternalInput")
    neigh_h = nc.dram_tensor("neigh_vecs", [n_nodes, K, D], FP, kind="ExternalInput")
    w_h = nc.dram_tensor("W", [D, O], FP, kind="ExternalInput")
    b_h = nc.dram_tensor("b", [O], FP, kind="ExternalInput")
    out_h = nc.dram_tensor("out", [n_nodes, O], FP, kind="ExternalOutput")

    n_dc = D // P  # d-chunks for the GEMM contraction

    with tile.TileContext(nc) as tc:
        with (
            tc.tile_pool(name="const", bufs=1) as const_pool,
            tc.tile_pool(name="neigh", bufs=neigh_bufs) as neigh_pool,
            tc.tile_pool(name="small", bufs=3) as small_pool,
            tc.tile_pool(name="outp", bufs=3) as out_pool,
            tc.tile_pool(name="ps_t", bufs=2, space="PSUM") as ps_t_pool,
            tc.tile_pool(name="ps_o", bufs=2, space="PSUM") as ps_o_pool,
        ):
            # --- constants (w_sb/b_sb DMAs are emitted after tile 0's loads
            # below, so the neigh stream starts immediately on the ring; W is
            # only needed by the first GEMM at ~20us) ---
            # w_sb[p, c, o] = W[c*128 + p, o] -> chunk c is the rhs for d-chunk c
            w_sb = const_pool.tile([P, n_dc * O], FP)
            b_sb = const_pool.tile([1, O], FP)
            ident = const_pool.tile([P, P], FP)
            make_identity(nc, ident)
            ones = const_pool.tile([1, P], FP)
            nc.gpsimd.memset(ones, 1.0)

            def transpose_scaled(src):
                """PE-transpose src [n,d] into [d,n] chunks, scale by 1/(K+1)
                on the PSUM->SBUF copy."""
                tps = ps_t_pool.tile([P, D], FP, tag="tps", name="tps")
                for c in range(n_dc):
                    nc.tensor.transpose(
                        tps[:, bass.ts(c, P)], src[:, bass.ts(c, P)], ident
                    )
                t_sb = small_pool.tile([P, D], FP, tag="tsb", name="tsb")
                nc.scalar.activation(
                    t_sb, tps, mybir.ActivationFunctionType.Copy, scale=INV
                )
                return t_sb

            def gemm_acc(out_pss, sumT, start):
                for c in range(n_dc):
                    for oh in range(len(out_pss)):
                        nc.tensor.matmul(
                            out_pss[oh],
                            lhsT=sumT[:, bass.ts(c, P)],
                            rhs=w_sb[:, c * O + oh * 512 : c * O + oh * 512 + 512],
                            start=(start and c == 0),
                            stop=False,
                        )

            k1n = K // 2  # 12 neigh groups in half 1 (+ self = 13 groups)
            k2 = K - k1n  # 13 neigh groups in half 2
            for i in range(nt):
                # split the neigh load so the k-sum (DVE tree adds; these run
                # at model speed where tensor_reduce measured ~1.6x slower)
                # starts while the second half streams, and SBUF slots
                # release at half-tile granularity. self_vecs rides in half 1
                # as a 13th group so no separate add is needed.
                nh1 = neigh_pool.tile([P, (k1n + 1) * D], FP, tag="nh1", name="nh1")
                nc.sync.dma_start(nh1[:, : k1n * D], neigh_h[bass.ts(i, P), 0:k1n, :])
                nc.sync.dma_start(nh1[:, k1n * D :], self_h[bass.ts(i, P), :])
                nh2 = neigh_pool.tile([P, k2 * D], FP, tag="nh2", name="nh2")
                nc.sync.dma_start(nh2, neigh_h[bass.ts(i, P), k1n:K, :])
                if i == 0:
                    nc.sync.dma_start(
                        w_sb, w_h[:, :].rearrange("(c p) o -> p c o", p=P)
                    )
                    nc.sync.dma_start(b_sb, b_h[:])
                n_oh = O // 512

                def make_out_pss():
                    return [
                        ps_o_pool.tile(
                            [P, 512], FP, tag=f"out_ps{oh}", name=f"out_ps{oh}"
                        )
                        for oh in range(n_oh)
                    ]

                _tree_fold(nc, nh1, k1n + 1)
                _tree_fold(nc, nh2, k2)
                summ = small_pool.tile([P, D], FP)
                nc.vector.tensor_add(summ, nh1[:, :D], nh2[:, :D])
                sumT = transpose_scaled(summ)
                out_sb = out_pool.tile([P, O], FP)
                out_pss = make_out_pss()
                gemm_acc(out_pss, sumT, start=True)

                for oh in range(n_oh):
                    # bias via K=1 matmul: ones.T @ b broadcasts b over nodes
                    nc.tensor.matmul(
                        out_pss[oh],
                        lhsT=ones,
                        rhs=b_sb[:, bass.ts(oh, 512)],
                        start=False,
                        stop=True,
                    )
                    nc.scalar.activation(
                        out_sb[:, bass.ts(oh, 512)],
                        out_pss[oh],
                        mybir.ActivationFunctionType.Relu,
                    )
                nc.scalar.dma_start(out_h[bass.ts(i, P), :], out_sb)

    nc.compile()
    return nc


def shard_inputs(inputs: dict) -> list[dict]:
    n = inputs["self_vecs"].shape[0]
    per = n // N_CORES
    maps = []
    for c in range(N_CORES):
        sl = slice(c * per, (c + 1) * per)
        maps.append(
            {
                "self_vecs": np.ascontiguousarray(inputs["self_vecs"][sl], np.float32),
                "neigh_vecs": np.ascontiguousarray(
                    inputs["neigh_vecs"][sl], np.float32
                ),
                "W": np.ascontiguousarray(inputs["W"], np.float32),
                "b": np.ascontiguousarray(inputs["b"], np.float32),
            }
        )
    return maps


def run_sharded(inputs: dict, trace: bool = False, **kwargs):
    from concourse.bass_utils import run_bass_kernel_spmd

    in_maps = shard_inputs(inputs)
    n_nodes = in_maps[0]["self_vecs"].shape[0]
    nc = build_nc(n_nodes)
    res = run_bass_kernel_spmd(
        nc, in_maps, core_ids=list(range(N_CORES)), trace=trace, **kwargs
    )
    out = np.concatenate([res.results[c]["out"] for c in range(N_CORES)], axis=0)
    return out, res


def kernel(**inputs) -> np.ndarray:
    out, _ = run_sharded(inputs, trace=False)
    return out



# revision 8
# speedup vs baseline: 1.0670x; 1.0670x over previous
"""GCN aggregator kernel for Trainium2 (Bass/Tile), 8-core data-parallel.

Computes: out = relu(((sum_k neigh[:,k,:] + self) / (K+1)) @ W + b)
Sharding: nodes (N) split evenly across 8 NeuronCores; W/b replicated.

The kernel is HBM-read-bound (~105 MB of neigh per core), so all on-chip
compute runs in fp16: the SWDGE (gpsimd) DMA path casts fp32->fp16 in
flight at no HBM cost, which doubles DVE fold throughput (2x perf mode),
quadruples PE GEMM rate, and halves SBUF footprint (deeper pipelining).
PSUM accumulation stays fp32; the output is stored as fp32.

Per 128-node tile on each core:
  1. cast-DMA neigh tile [128, K*D] + self tile [128, D]      (gpsimd SWDGE)
  2. DVE pairwise-fold sum over k in fp16                     (VectorE)
  3. PE transpose sum -> sumT in PSUM, ACT copy w/ 1/(K+1)    (TensorE/ScalarE)
  4. PE GEMM sumT.T @ W accumulated over 4 d-chunks + bias    (TensorE)
  5. ACT relu PSUM->SBUF fp32, DMA store                      (ScalarE HWDGE)
"""

import os
import sys

import numpy as np

for _p in ("/opt/trn_rl_repo", "/root/.axon_site/_ro/trn_rl_repo"):
    if os.path.isdir(_p) and _p not in sys.path:
        sys.path.insert(0, _p)

import concourse.bass as bass
import concourse.tile as tile
from concourse import bacc, mybir
from concourse.masks import make_identity

N, K, D, O = 16384, 25, 512, 1024
N_CORES = 8
P = 128  # nodes per tile (partition count)
INV = 1.0 / (K + 1)
FP = mybir.dt.float32
CP = mybir.dt.float16  # on-chip compute dtype


def _tree_fold(nc, t, g):
    """In-place pairwise fold of `g` contiguous D-sized groups in tile t;
    result lands in t[:, :D]."""
    while g > 1:
        lo = g // 2
        nc.vector.tensor_add(
            t[:, : lo * D], t[:, : lo * D], t[:, (g - lo) * D : g * D]
        )
        g -= lo


def build_nc(n_nodes: int, neigh_bufs: int = 4) -> bass.Bass:
    """Build the per-core Bass program for a shard of `n_nodes` nodes."""
    assert n_nodes % P == 0
    nt = n_nodes // P

    nc = bacc.Bacc("TRN2", target_bir_lowering=False, debug=False)
    self_h = nc.dram_tensor("self_vecs", [n_nodes, D], FP, kind="ExternalInput")
    neigh_h = nc.dram_tensor("neigh_vecs", [n_nodes, K, D], FP, kind="ExternalInput")
    w_h = nc.dram_tensor("W", [D, O], FP, kind="ExternalInput")
    b_h = nc.dram_tensor("b", [O], FP, kind="ExternalInput")
    out_h = nc.dram_tensor("out", [n_nodes, O], FP, kind="ExternalOutput")

    n_dc = D // P  # d-chunks for the GEMM contraction

    with tile.TileContext(nc) as tc:
        with (
            tc.tile_pool(name="const", bufs=1) as const_pool,
            tc.tile_pool(name="neigh", bufs=neigh_bufs) as neigh_pool,
            tc.tile_pool(name="small", bufs=3) as small_pool,
            tc.tile_pool(name="outp", bufs=3) as out_pool,
            tc.tile_pool(name="ps_t", bufs=2, space="PSUM") as ps_t_pool,
            tc.tile_pool(name="ps_o", bufs=2, space="PSUM") as ps_o_pool,
        ):
            # --- constants (w_sb/b_sb DMAs are emitted after tile 0's loads
            # below, so the neigh stream starts immediately on the ring; W is
            # only needed by the first GEMM at ~20us) ---
            # w_sb[p, c, o] = W[c*128 + p, o] -> chunk c is the rhs for d-chunk c
            w_sb = const_pool.tile([P, n_dc * O], CP)
            b_sb = const_pool.tile([1, O], CP)
            ident = const_pool.tile([P, P], CP)
            make_identity(nc, ident)
            ones = const_pool.tile([1, P], CP)
            nc.gpsimd.memset(ones, 1.0)

            def transpose_scaled(src):
                """PE-transpose src [n,d] into [d,n] chunks, scale by 1/(K+1)
                on the PSUM->SBUF copy."""
                tps = ps_t_pool.tile([P, D], CP, tag="tps", name="tps")
                for c in range(n_dc):
                    nc.tensor.transpose(
                        tps[:, bass.ts(c, P)], src[:, bass.ts(c, P)], ident
                    )
                t_sb = small_pool.tile([P, D], CP, tag="tsb", name="tsb")
                nc.scalar.activation(
                    t_sb, tps, mybir.ActivationFunctionType.Copy, scale=INV
                )
                return t_sb

            def gemm_acc(out_pss, sumT, start):
                for c in range(n_dc):
                    for oh in range(len(out_pss)):
                        nc.tensor.matmul(
                            out_pss[oh],
                            lhsT=sumT[:, bass.ts(c, P)],
                            rhs=w_sb[:, c * O + oh * 512 : c * O + oh * 512 + 512],
                            start=(start and c == 0),
                            stop=False,
                        )

            k1n = K // 2  # 12 neigh groups in half 1 (+ self = 13 groups)
            k2 = K - k1n  # 13 neigh groups in half 2
            for i in range(nt):
                # split the neigh load so the k-sum (DVE tree adds; these run
                # at model speed where tensor_reduce measured ~1.6x slower)
                # starts while the second half streams, and SBUF slots
                # release at half-tile granularity. self_vecs rides in half 1
                # as a 13th group so no separate add is needed.
                nh1 = neigh_pool.tile([P, (k1n + 1) * D], CP, tag="nh1", name="nh1")
                nc.gpsimd.dma_start(nh1[:, : k1n * D], neigh_h[bass.ts(i, P), 0:k1n, :])
                nc.gpsimd.dma_start(nh1[:, k1n * D :], self_h[bass.ts(i, P), :])
                nh2 = neigh_pool.tile([P, k2 * D], CP, tag="nh2", name="nh2")
                nc.gpsimd.dma_start(nh2, neigh_h[bass.ts(i, P), k1n:K, :])
                if i == 0:
                    nc.gpsimd.dma_start(
                        w_sb, w_h[:, :].rearrange("(c p) o -> p c o", p=P)
                    )
                    nc.gpsimd.dma_start(b_sb, b_h[:])
                n_oh = O // 512

                def make_out_pss():
                    return [
                        ps_o_pool.tile(
                            [P, 512], FP, tag=f"out_ps{oh}", name=f"out_ps{oh}"
                        )
                        for oh in range(n_oh)
                    ]

                _tree_fold(nc, nh1, k1n + 1)
                _tree_fold(nc, nh2, k2)
                summ = small_pool.tile([P, D], CP)
                nc.vector.tensor_add(summ, nh1[:, :D], nh2[:, :D])
                sumT = transpose_scaled(summ)
                out_sb = out_pool.tile([P, O], FP)
                out_pss = make_out_pss()
                gemm_acc(out_pss, sumT, start=True)

                for oh in range(n_oh):
                    # bias via K=1 matmul: ones.T @ b broadcasts b over nodes
                    nc.tensor.matmul(
                        out_pss[oh],
                        lhsT=ones,
                        rhs=b_sb[:, bass.ts(oh, 512)],
                        start=False,
                        stop=True,
                    )
                    nc.scalar.activation(
                        out_sb[:, bass.ts(oh, 512)],
                        out_pss[oh],
                        mybir.ActivationFunctionType.Relu,
                    )
                nc.scalar.dma_start(out_h[bass.ts(i, P), :], out_sb)

    nc.compile()
    return nc


def shard_inputs(inputs: dict) -> list[dict]:
    n = inputs["self_vecs"].shape[0]
    per = n // N_CORES
    maps = []
    for c in range(N_CORES):
        sl = slice(c * per, (c + 1) * per)
        maps.append(
            {
                "self_vecs": np.ascontiguousarray(inputs["self_vecs"][sl], np.float32),
                "neigh_vecs": np.ascontiguousarray(
                    inputs["neigh_vecs"][sl], np.float32
                ),
                "W": np.ascontiguousarray(inputs["W"], np.float32),
                "b": np.ascontiguousarray(inputs["b"], np.float32),
            }
        )
    return maps


def run_sharded(inputs: dict, trace: bool = False, **kwargs):
    from concourse.bass_utils import run_bass_kernel_spmd

    in_maps = shard_inputs(inputs)
    n_nodes = in_maps[0]["self_vecs"].shape[0]
    nc = build_nc(n_nodes)
    res = run_bass_kernel_spmd(
        nc, in_maps, core_ids=list(range(N_CORES)), trace=trace, **kwargs
    )
    out = np.concatenate([res.results[c]["out"] for c in range(N_CORES)], axis=0)
    return out, res


def kernel(**inputs) -> np.ndarray:
    out, _ = run_sharded(inputs, trace=False)
    return out



# revision 12
# speedup vs baseline: 1.0695x; 1.0023x over previous
"""GCN aggregator kernel for Trainium2 (Bass/Tile), 8-core data-parallel.

Computes: out = relu(((sum_k neigh[:,k,:] + self) / (K+1)) @ W + b)
Sharding: nodes (N) split evenly across 8 NeuronCores; W/b replicated.

The kernel is HBM-read-bound (~105 MB of neigh per core). Loads stay on
the HWDGE (sync) path in fp32 — the SWDGE cast path has a straggler SDMA
engine (15) that serializes ~40us of stream time. The fp32->fp16 cast
happens in the first DVE fold level instead (wide pair-add, fp32 in,
fp16 out); the rest of the fold runs fp16 at 2x DVE rate and the GEMM
runs fp16 at 4x PE rate. PSUM stays fp32; output is stored fp32.

Per 128-node tile on each core:
  1. DMA nh1 = 19 neigh groups + self [128, 20*D] fp32, then
     nh2 = 6 neigh groups [128, 6*D] fp32                     (sync HWDGE)
  2. DVE L1: h[:, :10D] = nh1 lo+hi (fp32->fp16), h[:, 10D:13D] =
     nh2 lo+hi; L2: fp16 tree-fold of 13 groups               (VectorE)
  3. PE transpose sum -> sumT in PSUM, ACT copy w/ 1/(K+1)    (TensorE/ScalarE)
  4. PE GEMM sumT.T @ W accumulated over 4 d-chunks + bias    (TensorE)
  5. ACT relu PSUM->SBUF fp32, DMA store                      (ScalarE HWDGE)
"""

import os
import sys

import numpy as np

for _p in ("/opt/trn_rl_repo", "/root/.axon_site/_ro/trn_rl_repo"):
    if os.path.isdir(_p) and _p not in sys.path:
        sys.path.insert(0, _p)

import concourse.bass as bass
import concourse.tile as tile
from concourse import bacc, mybir
from concourse.masks import make_identity

N, K, D, O = 16384, 25, 512, 1024
N_CORES = 8
P = 128  # nodes per tile (partition count)
INV = 1.0 / (K + 1)
FP = mybir.dt.float32
CP = mybir.dt.float16  # on-chip compute dtype


def _tree_fold(nc, t, g):
    """In-place pairwise fold of `g` contiguous D-sized groups in tile t;
    result lands in t[:, :D]."""
    while g > 1:
        lo = g // 2
        nc.vector.tensor_add(
            t[:, : lo * D], t[:, : lo * D], t[:, (g - lo) * D : g * D]
        )
        g -= lo


def build_nc(n_nodes: int, neigh_bufs: int = 2) -> bass.Bass:
    """Build the per-core Bass program for a shard of `n_nodes` nodes."""
    assert n_nodes % P == 0
    nt = n_nodes // P

    nc = bacc.Bacc("TRN2", target_bir_lowering=False, debug=False)
    self_h = nc.dram_tensor("self_vecs", [n_nodes, D], FP, kind="ExternalInput")
    neigh_h = nc.dram_tensor("neigh_vecs", [n_nodes, K, D], FP, kind="ExternalInput")
    w_h = nc.dram_tensor("W", [D, O], FP, kind="ExternalInput")
    b_h = nc.dram_tensor("b", [O], FP, kind="ExternalInput")
    out_h = nc.dram_tensor("out", [n_nodes, O], FP, kind="ExternalOutput")

    n_dc = D // P  # d-chunks for the GEMM contraction

    with tile.TileContext(nc) as tc:
        with (
            tc.tile_pool(name="const", bufs=1) as const_pool,
            tc.tile_pool(name="neigh", bufs=neigh_bufs) as neigh_pool,
            tc.tile_pool(name="hsum", bufs=3) as h_pool,
            tc.tile_pool(name="small", bufs=3) as small_pool,
            tc.tile_pool(name="outp", bufs=3) as out_pool,
            tc.tile_pool(name="ps_t", bufs=2, space="PSUM") as ps_t_pool,
            tc.tile_pool(name="ps_o", bufs=2, space="PSUM") as ps_o_pool,
        ):
            # --- constants (w_sb/b_sb DMAs are emitted after tile 0's loads
            # below, so the neigh stream starts immediately on the ring; W is
            # only needed by the first GEMM at ~20us) ---
            # w_sb[p, c, o] = W[c*128 + p, o] -> chunk c is the rhs for d-chunk c
            w_sb = const_pool.tile([P, n_dc * O], CP)
            b_sb = const_pool.tile([1, O], CP)
            ident = const_pool.tile([P, P], CP)
            make_identity(nc, ident)
            ones = const_pool.tile([1, P], CP)
            nc.gpsimd.memset(ones, 1.0)

            def transpose_scaled(src):
                """PE-transpose src [n,d] into [d,n] chunks, scale by 1/(K+1)
                on the PSUM->SBUF copy."""
                tps = ps_t_pool.tile([P, D], CP, tag="tps", name="tps")
                for c in range(n_dc):
                    nc.tensor.transpose(
                        tps[:, bass.ts(c, P)], src[:, bass.ts(c, P)], ident
                    )
                t_sb = small_pool.tile([P, D], CP, tag="tsb", name="tsb")
                nc.scalar.activation(
                    t_sb, tps, mybir.ActivationFunctionType.Copy, scale=INV
                )
                return t_sb

            def gemm_acc(out_pss, sumT, start):
                for c in range(n_dc):
                    for oh in range(len(out_pss)):
                        nc.tensor.matmul(
                            out_pss[oh],
                            lhsT=sumT[:, bass.ts(c, P)],
                            rhs=w_sb[:, c * O + oh * 512 : c * O + oh * 512 + 512],
                            start=(start and c == 0),
                            stop=False,
                        )

            k1n = 19  # neigh groups in chunk 1 (+ self = 20 groups)
            k2 = K - k1n  # 6 neigh groups in chunk 2 (small -> short tail)
            g1 = (k1n + 1) // 2  # 10 fp16 groups from chunk 1's pair-add
            g2 = k2 // 2  # 3 fp16 groups from chunk 2's pair-add
            for i in range(nt):
                # chunk 1 (19 neigh + self as 20th group) streams first; its
                # fp32->fp16 pair-add (the cast) runs while chunk 2 streams.
                # Chunk 2 is kept small so the last-landing data needs little
                # DVE work -> short drain tail after the stream ends.
                nh1 = neigh_pool.tile([P, (k1n + 1) * D], FP, tag="nh1", name="nh1")
                nc.sync.dma_start(nh1[:, : k1n * D], neigh_h[bass.ts(i, P), 0:k1n, :])
                nc.sync.dma_start(nh1[:, k1n * D :], self_h[bass.ts(i, P), :])
                nh2 = neigh_pool.tile([P, k2 * D], FP, tag="nh2", name="nh2")
                nc.sync.dma_start(nh2, neigh_h[bass.ts(i, P), k1n:K, :])
                if i == 0:
                    nc.gpsimd.dma_start(
                        w_sb, w_h[:, :].rearrange("(c p) o -> p c o", p=P)
                    )
                    nc.gpsimd.dma_start(b_sb, b_h[:])
                n_oh = O // 512

                def make_out_pss():
                    return [
                        ps_o_pool.tile(
                            [P, 512], FP, tag=f"out_ps{oh}", name=f"out_ps{oh}"
                        )
                        for oh in range(n_oh)
                    ]

                # L1 pair-adds: fp32 inputs, fp16 output (the cast rides the
                # first fold level at no extra DVE cost).
                h = h_pool.tile([P, (g1 + g2) * D], CP, tag="h", name="h")
                nc.vector.tensor_add(
                    h[:, : g1 * D], nh1[:, : g1 * D], nh1[:, g1 * D : 2 * g1 * D]
                )
                nc.vector.tensor_add(
                    h[:, g1 * D :], nh2[:, : g2 * D], nh2[:, g2 * D : 2 * g2 * D]
                )
                # L2: fp16 tree-fold of the 13 groups
                _tree_fold(nc, h, g1 + g2)
                sumT = transpose_scaled(h[:, :D])
                out_sb = out_pool.tile([P, O], FP)
                out_pss = make_out_pss()
                gemm_acc(out_pss, sumT, start=True)

                for oh in range(n_oh):
                    # bias via K=1 matmul: ones.T @ b broadcasts b over nodes
                    nc.tensor.matmul(
                        out_pss[oh],
                        lhsT=ones,
                        rhs=b_sb[:, bass.ts(oh, 512)],
                        start=False,
                        stop=True,
                    )
                    nc.scalar.activation(
                        out_sb[:, bass.ts(oh, 512)],
                        out_pss[oh],
                        mybir.ActivationFunctionType.Relu,
                    )
                nc.scalar.dma_start(out_h[bass.ts(i, P), :], out_sb)

    nc.compile()
    return nc


def shard_inputs(inputs: dict) -> list[dict]:
    n = inputs["self_vecs"].shape[0]
    per = n // N_CORES
    maps = []
    for c in range(N_CORES):
        sl = slice(c * per, (c + 1) * per)
        maps.append(
            {
                "self_vecs": np.ascontiguousarray(inputs["self_vecs"][sl], np.float32),
                "neigh_vecs": np.ascontiguousarray(
                    inputs["neigh_vecs"][sl], np.float32
                ),
                "W": np.ascontiguousarray(inputs["W"], np.float32),
                "b": np.ascontiguousarray(inputs["b"], np.float32),
            }
        )
    return maps


def run_sharded(inputs: dict, trace: bool = False, **kwargs):
    from concourse.bass_utils import run_bass_kernel_spmd

    in_maps = shard_inputs(inputs)
    n_nodes = in_maps[0]["self_vecs"].shape[0]
    nc = build_nc(n_nodes)
    res = run_bass_kernel_spmd(
        nc, in_maps, core_ids=list(range(N_CORES)), trace=trace, **kwargs
    )
    out = np.concatenate([res.results[c]["out"] for c in range(N_CORES)], axis=0)
    return out, res


def kernel(**inputs) -> np.ndarray:
    out, _ = run_sharded(inputs, trace=False)
    return out

